# revision 6
# baseline (speedup 1.0000x reference)
"""Bidirectional LSTM over embedded event ids — Trainium2 Bass kernel.

Problem shapes (hardcoded): ids [32,64,256] int32, embed [6000,64],
per-direction LSTM E=H=64, output [32,64,256,128] f32.

Strategy: direction-parallel + data-parallel. Cores 0-3 run the forward
LSTM on sequence quarters 0-3 (512 seqs each); cores 4-7 run the backward
LSTM on the same quarters (host pre-reverses time). Per core, the 512
sequences are packed two-per-partition-lane: partition p = hdim + 64*(s
>= 256), free column j = s % 256. All per-step tensors are [128, 256]
with full partition utilization.

The 256 free columns split into G=3 interleaved groups (86/85/85) whose
recurrence chains overlap on the engines — the serial chain (matmul ->
sigmoid -> cell update -> tanh -> h-mul) of one group hides behind the
activation work of the other two.

Per group per step:
  - 8 matmuls (4 gates x {x-part, h-part}), bf16, accumulate in PSUM
    Z[128, 4f]; weights are [128,128] blockdiag(Wq, Wq) so both
    sequence halves share one matmul. g-gate weights pre-scaled by 2.
  - one Sigmoid over all 4 gates: S = sigma(Z)   (tanh(zg) = 2*sig(2 zg)-1)
  - t1 = (S_g - 0.5) * S_i           (scalar_tensor_tensor, Pool)
  - t2 = S_f * c                     (tensor_tensor, DVE)
  - c  = 2*t1 + t2                   (scalar_tensor_tensor, DVE)
  - Tc = tanh(c)                     (Act; same act table as sigmoid)
  - h  = Tc * S_o -> bf16 h-ring     (tensor_tensor, DVE)

x is streamed from HBM in T-step blocks into a 3T-slot bf16 ring; h is
written to a 2T-slot bf16 ring that doubles as matmul rhs source and
output staging (one batched DMA per T steps each way).
"""

import numpy as np
import ml_dtypes

B, S, L, E, H, V = 32, 64, 256, 64, 64, 6000
NCORES = 8
NSEQ = B * S               # 2048
NC_SEQ = 512               # sequences per core (one direction)
NQ = NSEQ // NC_SEQ        # 4 sequence quarters
COLS = NC_SEQ // 2         # 256 free columns (2 seqs per partition lane)
GB = [0, 86, 171, 256]     # group boundaries over free columns
NG = 3
T = 16                     # DMA block (timesteps)
RX = 3 * T                 # x ring slots
RH = 2 * T                 # h ring slots
NB = L // T

_CACHE = {}
_BF16 = ml_dtypes.bfloat16


def _build(with_bias):
    import concourse.bacc as bacc
    import concourse.tile as tile
    from concourse import mybir

    dt = mybir.dt
    AF = mybir.ActivationFunctionType
    OP = mybir.AluOpType

    nc = bacc.Bacc("TRN2", num_devices=NCORES, debug=False)
    xc_d = nc.dram_tensor("xc", (128, L * COLS), dt.bfloat16,
                          kind="ExternalInput")
    wx_d = nc.dram_tensor("wx", (128, 512), dt.bfloat16, kind="ExternalInput")
    wh_d = nc.dram_tensor("wh", (128, 512), dt.bfloat16, kind="ExternalInput")
    if with_bias:
        wb_d = nc.dram_tensor("wb", (128, 512), dt.bfloat16,
                              kind="ExternalInput")
    out_d = nc.dram_tensor("out", (128, L * COLS), dt.bfloat16,
                           kind="ExternalOutput")

    with tile.TileContext(nc) as tc:
        with (
            tc.tile_pool(name="big", bufs=1) as big,
            tc.tile_pool(name="zp", bufs=2, space="PSUM") as zp,
        ):
            XR = big.tile([128, RX * COLS], dt.bfloat16, name="XR", tag="XR")
            HR = big.tile([128, RH * COLS], dt.bfloat16, name="HR", tag="HR")
            wxt = big.tile([128, 512], dt.bfloat16, name="wxt", tag="wxt")
            wht = big.tile([128, 512], dt.bfloat16, name="wht", tag="wht")
            nc.sync.dma_start(out=wxt[:, :], in_=wx_d.ap())
            nc.sync.dma_start(out=wht[:, :], in_=wh_d.ap())
            if with_bias:
                wbt = big.tile([128, 512], dt.bfloat16, name="wbt", tag="wbt")
                nc.sync.dma_start(out=wbt[:, :], in_=wb_d.ap())
                ones = big.tile([128, COLS], dt.bfloat16, name="ones",
                                tag="ones")
                nc.vector.memset(ones[:, :], 1.0)
            FS = [GB[g + 1] - GB[g] for g in range(NG)]
            S_t, c_t, Tc_t, t1_t, t2_t = {}, {}, {}, {}, {}
            for g in range(NG):
                f = FS[g]
                S_t[g] = big.tile([128, 4 * f], dt.float32, name=f"S{g}",
                                  tag=f"S{g}")
                c_t[g] = big.tile([128, f], dt.float32, name=f"c{g}",
                                  tag=f"c{g}")
                nc.vector.memset(c_t[g][:, :], 0.0)
                Tc_t[g] = big.tile([128, f], dt.float32, name=f"Tc{g}",
                                   tag=f"Tc{g}")
                t1_t[g] = big.tile([128, f], dt.float32, name=f"t1{g}",
                                   tag=f"t1{g}")
                t2_t[g] = big.tile([128, f], dt.float32, name=f"t2{g}",
                                   tag=f"t2{g}")

            def xdma(b):
                lo = (b % 3) * T * COLS
                nc.sync.dma_start(out=XR[:, lo:lo + T * COLS],
                                  in_=xc_d.ap()[:, b * T * COLS:
                                                (b + 1) * T * COLS])

            for b in range(min(3, NB)):
                xdma(b)

            # Software-pipelined emission over virtual slots v = 3t + g.
            # Per slot: matmuls+gate-sigmoid for (g,t), then the cell-update
            # chain and tanh/h for the PREVIOUS slot, so the activation queue
            # alternates [A1_v, A2_{v-1}] and each group's serial chain hides
            # behind the other two groups' engine work.
            zt = {}

            def emit_mm_a1(v):
                g, t = v % NG, v // NG
                f, a = FS[g], GB[g]
                z = zp.tile([128, 4 * f], dt.float32, name=f"z{g}",
                            tag=f"z{g}")
                zt[v] = z
                xs = (t % RX) * COLS
                # start=True only on the first matmul into the psum tile:
                # start marks the whole 2KB zero-region pending-zero, so
                # later writes overwrite-as-zero once and then accumulate.
                rhs = XR[:, xs + a:xs + a + f]
                for q in range(4):
                    last = (t == 0) and not with_bias and q == 3
                    nc.tensor.matmul(z[:, q * f:(q + 1) * f],
                                     wxt[:, q * 128:(q + 1) * 128],
                                     rhs, start=(q == 0), stop=last)
                if with_bias:
                    for q in range(4):
                        nc.tensor.matmul(z[:, q * f:(q + 1) * f],
                                         wbt[:, q * 128:(q + 1) * 128],
                                         ones[:, a:a + f], start=False,
                                         stop=(t == 0) and q == 3)
                if t > 0:
                    hp = ((t - 1) % RH) * COLS
                    rhs = HR[:, hp + a:hp + a + f]
                    for q in range(4):
                        nc.tensor.matmul(z[:, q * f:(q + 1) * f],
                                         wht[:, q * 128:(q + 1) * 128],
                                         rhs, start=False, stop=(q == 3))
                nc.scalar.activation(S_t[g][:, :], z[:, :], AF.Sigmoid)

            def emit_cell(v):
                g, t = v % NG, v // NG
                f, a = FS[g], GB[g]
                nc.gpsimd.tensor_tensor(t2_t[g][:, :], S_t[g][:, f:2 * f],
                                        c_t[g][:, :], OP.mult)
                nc.vector.scalar_tensor_tensor(
                    out=t1_t[g][:, :], in0=S_t[g][:, 2 * f:3 * f],
                    scalar=0.5, in1=S_t[g][:, 0:f],
                    op0=OP.subtract, op1=OP.mult)
                nc.vector.scalar_tensor_tensor(
                    out=c_t[g][:, :], in0=t1_t[g][:, :], scalar=2.0,
                    in1=t2_t[g][:, :], op0=OP.mult, op1=OP.add)
                nc.scalar.activation(Tc_t[g][:, :], c_t[g][:, :], AF.Tanh)
                hs = (t % RH) * COLS
                nc.vector.tensor_tensor(HR[:, hs + a:hs + a + f],
                                        Tc_t[g][:, :],
                                        S_t[g][:, 3 * f:4 * f], OP.mult)
                del zt[v]
                # x prefetch / h writeback bookkeeping on group-2 boundaries
                if g == 0 and t % T == 1 and t // T >= 1 and t // T + 2 < NB:
                    xdma(t // T + 2)
                if g == NG - 1 and t % T == T - 1:
                    k = t // T
                    lo = (k % 2) * T * COLS
                    nc.sync.dma_start(
                        out=out_d.ap()[:, k * T * COLS:(k + 1) * T * COLS],
                        in_=HR[:, lo:lo + T * COLS])

            # Pace the virtual slots (700ns each — just under the natural
            # pace) so the scheduler starts the groups staggered instead of
            # falling into the slower lockstep fixed point.
            for v in range(NG * L):
                with tc.tile_wait_until(v * 7e-4):
                    emit_mm_a1(v)
                    if v >= 1:
                        emit_cell(v - 1)
            emit_cell(NG * L - 1)

    nc.compile()
    return nc


def _get_nc(with_bias):
    key = ("v6", with_bias)
    if key not in _CACHE:
        _CACHE[key] = _build(with_bias)
    return _CACHE[key]


def _prep_w(Wk, Wr, b):
    """Blockdiag-packed lhsT weights [128, 4*128] bf16 for x- and h-parts,
    plus optional rank-1 bias lhsT. Gate order i,f,g,o; g pre-scaled by 2
    (tanh via sigmoid)."""
    Wk = np.asarray(Wk, np.float32)
    Wr = np.asarray(Wr, np.float32)
    b = np.asarray(b, np.float32)
    wx = np.zeros((128, 512), np.float32)
    wh = np.zeros((128, 512), np.float32)
    wb = np.zeros((128, 512), np.float32)
    for q in range(4):
        sc = 2.0 if q == 2 else 1.0
        Wq = Wk[:, q * 64:(q + 1) * 64] * sc
        Rq = Wr[:, q * 64:(q + 1) * 64] * sc
        wx[0:64, q * 128 + 0:q * 128 + 64] = Wq
        wx[64:128, q * 128 + 64:q * 128 + 128] = Wq
        wh[0:64, q * 128 + 0:q * 128 + 64] = Rq
        wh[64:128, q * 128 + 64:q * 128 + 128] = Rq
        bq = b[q * 64:(q + 1) * 64] * sc
        wb[0, q * 128 + 0:q * 128 + 64] = bq
        wb[0, q * 128 + 64:q * 128 + 128] = bq
    with_bias = bool(np.any(b != 0.0))
    return (wx.astype(_BF16), wh.astype(_BF16), wb.astype(_BF16), with_bias)


def _pack_x(ids_q, emb, rev):
    """ids_q [512, L] -> packed [128, L*COLS] bf16 (p = hdim + 64*(s>=256))."""
    x = emb[ids_q]                                   # [512, L, E] f32
    if rev:
        x = x[:, ::-1, :]
    xr = x.reshape(2, COLS, L, E).transpose(0, 3, 2, 1)   # [2, E, L, COLS]
    return np.ascontiguousarray(xr).reshape(128, L * COLS).astype(_BF16)


def _unpack_h(o, rev):
    """[128, L*COLS] -> h [512, L, 64] f32."""
    o = np.asarray(o, np.float32).reshape(2, 64, L, COLS)
    h = o.transpose(0, 3, 2, 1).reshape(NC_SEQ, L, 64)
    if rev:
        h = h[:, ::-1, :]
    return h


def _in_maps(ids, embed_table, Wk_f, Wr_f, b_f, Wk_b, Wr_b, b_b):
    ids2 = np.asarray(ids).reshape(NSEQ, L)
    emb = np.asarray(embed_table, dtype=np.float32)
    wx_f, wh_f, wb_f, bias_f = _prep_w(Wk_f, Wr_f, b_f)
    wx_b, wh_b, wb_b, bias_b = _prep_w(Wk_b, Wr_b, b_b)
    with_bias = bias_f or bias_b
    in_maps = []
    for m in range(NCORES):
        rev = m >= NQ
        q = m % NQ
        ids_q = ids2[q * NC_SEQ:(q + 1) * NC_SEQ]
        im = {"xc": _pack_x(ids_q, emb, rev),
              "wx": wx_b if rev else wx_f,
              "wh": wh_b if rev else wh_f}
        if with_bias:
            im["wb"] = wb_b if rev else wb_f
        in_maps.append(im)
    return in_maps, with_bias


def kernel(ids, embed_table, Wk_f, Wr_f, b_f, Wk_b, Wr_b, b_b):
    from concourse import bass_utils

    in_maps, with_bias = _in_maps(ids, embed_table, Wk_f, Wr_f, b_f,
                                  Wk_b, Wr_b, b_b)
    nc = _get_nc(with_bias)
    res = bass_utils.run_bass_kernel_spmd(nc, in_maps,
                                          core_ids=list(range(NCORES)))

    out = np.empty((NSEQ, L, 2 * H), dtype=np.float32)
    for m in range(NCORES):
        rev = m >= NQ
        q = m % NQ
        h = _unpack_h(res.results[m]["out"], rev)
        sl = slice(q * NC_SEQ, (q + 1) * NC_SEQ)
        if rev:
            out[sl, :, H:2 * H] = h
        else:
            out[sl, :, 0:H] = h
    return out.reshape(B, S, L, 2 * H)


# revision 7
# speedup vs baseline: 1.0060x; 1.0060x over previous
"""Bidirectional LSTM over embedded event ids — Trainium2 Bass kernel.

Problem shapes (hardcoded): ids [32,64,256] int32, embed [6000,64],
per-direction LSTM E=H=64, output [32,64,256,128] f32.

Strategy: direction-parallel + data-parallel. Cores 0-3 run the forward
LSTM on sequence quarters 0-3 (512 seqs each); cores 4-7 run the backward
LSTM on the same quarters (host pre-reverses time). Per core, the 512
sequences are packed two-per-partition-lane: partition p = hdim + 64*(s
>= 256), free column j = s % 256. All per-step tensors are [128, 256]
with full partition utilization.

The 256 free columns split into G=3 interleaved groups (86/85/85) whose
recurrence chains overlap on the engines — the serial chain (matmul ->
sigmoid -> cell update -> tanh -> h-mul) of one group hides behind the
activation work of the other two.

Per group per step:
  - 8 matmuls (4 gates x {x-part, h-part}), bf16, accumulate in PSUM
    Z[128, 4f]; weights are [128,128] blockdiag(Wq, Wq) so both
    sequence halves share one matmul. g-gate weights pre-scaled by 2.
  - one Sigmoid over all 4 gates: S = sigma(Z)   (tanh(zg) = 2*sig(2 zg)-1)
  - t1 = (S_g - 0.5) * S_i           (scalar_tensor_tensor, Pool)
  - t2 = S_f * c                     (tensor_tensor, DVE)
  - c  = 2*t1 + t2                   (scalar_tensor_tensor, DVE)
  - Tc = tanh(c)                     (Act; same act table as sigmoid)
  - h  = Tc * S_o -> bf16 h-ring     (tensor_tensor, DVE)

x is streamed from HBM in T-step blocks into a 3T-slot bf16 ring; h is
written to a 2T-slot bf16 ring that doubles as matmul rhs source and
output staging (one batched DMA per T steps each way).
"""

import numpy as np
import ml_dtypes

B, S, L, E, H, V = 32, 64, 256, 64, 64, 6000
NCORES = 8
NSEQ = B * S               # 2048
NC_SEQ = 512               # sequences per core (one direction)
NQ = NSEQ // NC_SEQ        # 4 sequence quarters
COLS = NC_SEQ // 2         # 256 free columns (2 seqs per partition lane)
GB = [0, 86, 171, 256]     # group boundaries over free columns
NG = 3
T = 16                     # DMA block (timesteps)
RX = 3 * T                 # x ring slots
RH = 2 * T                 # h ring slots
NB = L // T

_CACHE = {}
_BF16 = ml_dtypes.bfloat16


def _build(with_bias):
    import concourse.bacc as bacc
    import concourse.tile as tile
    from concourse import mybir

    dt = mybir.dt
    AF = mybir.ActivationFunctionType
    OP = mybir.AluOpType

    nc = bacc.Bacc("TRN2", num_devices=NCORES, debug=False)
    xc_d = nc.dram_tensor("xc", (128, L * COLS), dt.bfloat16,
                          kind="ExternalInput")
    wx_d = nc.dram_tensor("wx", (128, 512), dt.bfloat16, kind="ExternalInput")
    wh_d = nc.dram_tensor("wh", (128, 512), dt.bfloat16, kind="ExternalInput")
    if with_bias:
        wb_d = nc.dram_tensor("wb", (128, 512), dt.bfloat16,
                              kind="ExternalInput")
    out_d = nc.dram_tensor("out", (128, L * COLS), dt.bfloat16,
                           kind="ExternalOutput")

    with tile.TileContext(nc) as tc:
        with (
            tc.tile_pool(name="big", bufs=1) as big,
            tc.tile_pool(name="zp", bufs=2, space="PSUM") as zp,
        ):
            XR = big.tile([128, RX * COLS], dt.bfloat16, name="XR", tag="XR")
            HR = big.tile([128, RH * COLS], dt.bfloat16, name="HR", tag="HR")
            wxt = big.tile([128, 512], dt.bfloat16, name="wxt", tag="wxt")
            wht = big.tile([128, 512], dt.bfloat16, name="wht", tag="wht")
            nc.sync.dma_start(out=wxt[:, :], in_=wx_d.ap())
            nc.sync.dma_start(out=wht[:, :], in_=wh_d.ap())
            if with_bias:
                wbt = big.tile([128, 512], dt.bfloat16, name="wbt", tag="wbt")
                nc.sync.dma_start(out=wbt[:, :], in_=wb_d.ap())
                ones = big.tile([128, COLS], dt.bfloat16, name="ones",
                                tag="ones")
                nc.vector.memset(ones[:, :], 1.0)
            FS = [GB[g + 1] - GB[g] for g in range(NG)]
            S_t, c_t, Tc_t, t1_t, t2_t = {}, {}, {}, {}, {}
            for g in range(NG):
                f = FS[g]
                S_t[g] = big.tile([128, 4 * f], dt.float32, name=f"S{g}",
                                  tag=f"S{g}")
                c_t[g] = big.tile([128, f], dt.float32, name=f"c{g}",
                                  tag=f"c{g}")
                nc.vector.memset(c_t[g][:, :], 0.0)
                Tc_t[g] = big.tile([128, f], dt.float32, name=f"Tc{g}",
                                   tag=f"Tc{g}")
                t1_t[g] = big.tile([128, f], dt.float32, name=f"t1{g}",
                                   tag=f"t1{g}")
                t2_t[g] = big.tile([128, f], dt.float32, name=f"t2{g}",
                                   tag=f"t2{g}")

            def xdma(b):
                lo = (b % 3) * T * COLS
                nc.sync.dma_start(out=XR[:, lo:lo + T * COLS],
                                  in_=xc_d.ap()[:, b * T * COLS:
                                                (b + 1) * T * COLS])

            for b in range(min(3, NB)):
                xdma(b)

            # Software-pipelined emission over virtual slots v = 3t + g.
            # Per slot: matmuls+gate-sigmoid for (g,t), then the cell-update
            # chain and tanh/h for the PREVIOUS slot, so the activation queue
            # alternates [A1_v, A2_{v-1}] and each group's serial chain hides
            # behind the other two groups' engine work.
            zt = {}

            def emit_mm_a1(v):
                g, t = v % NG, v // NG
                f, a = FS[g], GB[g]
                z = zp.tile([128, 4 * f], dt.float32, name=f"z{g}",
                            tag=f"z{g}")
                zt[v] = z
                xs = (t % RX) * COLS
                # start=True only on the first matmul into the psum tile:
                # start marks the whole 2KB zero-region pending-zero, so
                # later writes overwrite-as-zero once and then accumulate.
                rhs = XR[:, xs + a:xs + a + f]
                for q in range(4):
                    last = (t == 0) and not with_bias and q == 3
                    nc.tensor.matmul(z[:, q * f:(q + 1) * f],
                                     wxt[:, q * 128:(q + 1) * 128],
                                     rhs, start=(q == 0), stop=last)
                if with_bias:
                    for q in range(4):
                        nc.tensor.matmul(z[:, q * f:(q + 1) * f],
                                         wbt[:, q * 128:(q + 1) * 128],
                                         ones[:, a:a + f], start=False,
                                         stop=(t == 0) and q == 3)
                if t > 0:
                    hp = ((t - 1) % RH) * COLS
                    rhs = HR[:, hp + a:hp + a + f]
                    for q in range(4):
                        nc.tensor.matmul(z[:, q * f:(q + 1) * f],
                                         wht[:, q * 128:(q + 1) * 128],
                                         rhs, start=False, stop=(q == 3))
                nc.scalar.activation(S_t[g][:, :], z[:, :], AF.Sigmoid)

            def emit_cell(v):
                g, t = v % NG, v // NG
                f, a = FS[g], GB[g]
                nc.gpsimd.tensor_tensor(t2_t[g][:, :], S_t[g][:, f:2 * f],
                                        c_t[g][:, :], OP.mult)
                nc.vector.scalar_tensor_tensor(
                    out=t1_t[g][:, :], in0=S_t[g][:, 2 * f:3 * f],
                    scalar=0.5, in1=S_t[g][:, 0:f],
                    op0=OP.subtract, op1=OP.mult)
                nc.vector.scalar_tensor_tensor(
                    out=c_t[g][:, :], in0=t1_t[g][:, :], scalar=2.0,
                    in1=t2_t[g][:, :], op0=OP.mult, op1=OP.add)
                nc.scalar.activation(Tc_t[g][:, :], c_t[g][:, :], AF.Tanh)
                hs = (t % RH) * COLS
                nc.vector.tensor_tensor(HR[:, hs + a:hs + a + f],
                                        Tc_t[g][:, :],
                                        S_t[g][:, 3 * f:4 * f], OP.mult)
                del zt[v]
                # x prefetch / h writeback bookkeeping on group-2 boundaries
                if g == 0 and t % T == 1 and t // T >= 1 and t // T + 2 < NB:
                    xdma(t // T + 2)
                if g == NG - 1 and t % T == T - 1:
                    k = t // T
                    lo = (k % 2) * T * COLS
                    nc.sync.dma_start(
                        out=out_d.ap()[:, k * T * COLS:(k + 1) * T * COLS],
                        in_=HR[:, lo:lo + T * COLS])

            # Stagger the three groups' recurrence chains by ~1/3 period:
            # dummy pool ops give group g's first gate-sigmoid a real
            # dependency on group g-1's, so the greedy scheduler settles
            # into the staggered (pipelined) steady state instead of the
            # slower lockstep one. The dummy writes one column of S which
            # the group's own A1 fully overwrites.
            def stagger(v):
                g = v % NG
                if v in (1, 2):
                    nc.gpsimd.tensor_tensor(S_t[g][:, 0:1],
                                            S_t[g - 1][:, 0:1],
                                            S_t[g - 1][:, 0:1], OP.mult)

            for v in range(NG * L):
                stagger(v)
                emit_mm_a1(v)
                if v >= 1:
                    emit_cell(v - 1)
            emit_cell(NG * L - 1)

    nc.compile()
    return nc


def _get_nc(with_bias):
    key = ("v6", with_bias)
    if key not in _CACHE:
        _CACHE[key] = _build(with_bias)
    return _CACHE[key]


def _prep_w(Wk, Wr, b):
    """Blockdiag-packed lhsT weights [128, 4*128] bf16 for x- and h-parts,
    plus optional rank-1 bias lhsT. Gate order i,f,g,o; g pre-scaled by 2
    (tanh via sigmoid)."""
    Wk = np.asarray(Wk, np.float32)
    Wr = np.asarray(Wr, np.float32)
    b = np.asarray(b, np.float32)
    wx = np.zeros((128, 512), np.float32)
    wh = np.zeros((128, 512), np.float32)
    wb = np.zeros((128, 512), np.float32)
    for q in range(4):
        sc = 2.0 if q == 2 else 1.0
        Wq = Wk[:, q * 64:(q + 1) * 64] * sc
        Rq = Wr[:, q * 64:(q + 1) * 64] * sc
        wx[0:64, q * 128 + 0:q * 128 + 64] = Wq
        wx[64:128, q * 128 + 64:q * 128 + 128] = Wq
        wh[0:64, q * 128 + 0:q * 128 + 64] = Rq
        wh[64:128, q * 128 + 64:q * 128 + 128] = Rq
        bq = b[q * 64:(q + 1) * 64] * sc
        wb[0, q * 128 + 0:q * 128 + 64] = bq
        wb[0, q * 128 + 64:q * 128 + 128] = bq
    with_bias = bool(np.any(b != 0.0))
    return (wx.astype(_BF16), wh.astype(_BF16), wb.astype(_BF16), with_bias)


def _pack_x(ids_q, emb, rev):
    """ids_q [512, L] -> packed [128, L*COLS] bf16 (p = hdim + 64*(s>=256))."""
    x = emb[ids_q]                                   # [512, L, E] f32
    if rev:
        x = x[:, ::-1, :]
    xr = x.reshape(2, COLS, L, E).transpose(0, 3, 2, 1)   # [2, E, L, COLS]
    return np.ascontiguousarray(xr).reshape(128, L * COLS).astype(_BF16)


def _unpack_h(o, rev):
    """[128, L*COLS] -> h [512, L, 64] f32."""
    o = np.asarray(o, np.float32).reshape(2, 64, L, COLS)
    h = o.transpose(0, 3, 2, 1).reshape(NC_SEQ, L, 64)
    if rev:
        h = h[:, ::-1, :]
    return h


def _in_maps(ids, embed_table, Wk_f, Wr_f, b_f, Wk_b, Wr_b, b_b):
    ids2 = np.asarray(ids).reshape(NSEQ, L)
    emb = np.asarray(embed_table, dtype=np.float32)
    wx_f, wh_f, wb_f, bias_f = _prep_w(Wk_f, Wr_f, b_f)
    wx_b, wh_b, wb_b, bias_b = _prep_w(Wk_b, Wr_b, b_b)
    with_bias = bias_f or bias_b
    in_maps = []
    for m in range(NCORES):
        rev = m >= NQ
        q = m % NQ
        ids_q = ids2[q * NC_SEQ:(q + 1) * NC_SEQ]
        im = {"xc": _pack_x(ids_q, emb, rev),
              "wx": wx_b if rev else wx_f,
              "wh": wh_b if rev else wh_f}
        if with_bias:
            im["wb"] = wb_b if rev else wb_f
        in_maps.append(im)
    return in_maps, with_bias


def kernel(ids, embed_table, Wk_f, Wr_f, b_f, Wk_b, Wr_b, b_b):
    from concourse import bass_utils

    in_maps, with_bias = _in_maps(ids, embed_table, Wk_f, Wr_f, b_f,
                                  Wk_b, Wr_b, b_b)
    nc = _get_nc(with_bias)
    res = bass_utils.run_bass_kernel_spmd(nc, in_maps,
                                          core_ids=list(range(NCORES)))

    out = np.empty((NSEQ, L, 2 * H), dtype=np.float32)
    for m in range(NCORES):
        rev = m >= NQ
        q = m % NQ
        h = _unpack_h(res.results[m]["out"], rev)
        sl = slice(q * NC_SEQ, (q + 1) * NC_SEQ)
        if rev:
            out[sl, :, H:2 * H] = h
        else:
            out[sl, :, 0:H] = h
    return out.reshape(B, S, L, 2 * H)


# revision 10
# speedup vs baseline: 1.1140x; 1.1073x over previous
"""Bidirectional LSTM over embedded event ids — Trainium2 Bass kernel.

Problem shapes (hardcoded): ids [32,64,256] int32, embed [6000,64],
per-direction LSTM E=H=64, output [32,64,256,128] f32.

Strategy: direction-parallel + data-parallel. Cores 0-3 run the forward
LSTM on sequence quarters 0-3 (512 seqs each); cores 4-7 run the backward
LSTM on the same quarters (host pre-reverses time). Per core, the 512
sequences are packed two-per-partition-lane: partition p = hdim + 64*(s
>= 256), free column j = s % 256. All per-step tensors are [128, 256]
with full partition utilization.

The 256 free columns split into G=3 interleaved groups (86/85/85) whose
recurrence chains overlap on the engines — the serial chain (matmul ->
sigmoid -> cell update -> tanh -> h-mul) of one group hides behind the
activation work of the other two.

Per group per step:
  - 8 matmuls (4 gates x {x-part, h-part}), bf16, accumulate in PSUM
    Z[128, 4f]; weights are [128,128] blockdiag(Wq, Wq) so both
    sequence halves share one matmul. g-gate weights pre-scaled by 2.
  - one Sigmoid over all 4 gates: S = sigma(Z)   (tanh(zg) = 2*sig(2 zg)-1)
  - t2 = S_f * c                     (tensor_tensor, Pool)
  - t1 = (S_g - 0.5) * S_i           (scalar_tensor_tensor, DVE)
  - c  = 2*t1 + t2                   (scalar_tensor_tensor, DVE)
  - Tc = tanh(c)                     (Act; same act table as sigmoid)
  - h  = Tc * S_o -> bf16 h-ring     (tensor_tensor, Pool — keeping the
    late-phase multiply off the DVE queue avoids a period-doubling
    oscillation between the three group pipelines)

x is streamed from HBM in T-step blocks into a 3T-slot bf16 ring; h is
written to a 2T-slot bf16 ring that doubles as matmul rhs source and
output staging (one batched DMA per T steps each way).
"""

import numpy as np
import ml_dtypes

B, S, L, E, H, V = 32, 64, 256, 64, 64, 6000
NCORES = 8
NSEQ = B * S               # 2048
NC_SEQ = 512               # sequences per core (one direction)
NQ = NSEQ // NC_SEQ        # 4 sequence quarters
COLS = NC_SEQ // 2         # 256 free columns (2 seqs per partition lane)
GB = [0, 86, 171, 256]     # group boundaries over free columns
NG = 3
T = 16                     # DMA block (timesteps)
RX = 3 * T                 # x ring slots
RH = 2 * T                 # h ring slots
NB = L // T

_CACHE = {}
_BF16 = ml_dtypes.bfloat16


def _build(with_bias):
    import concourse.bacc as bacc
    import concourse.tile as tile
    from concourse import mybir

    dt = mybir.dt
    AF = mybir.ActivationFunctionType
    OP = mybir.AluOpType

    nc = bacc.Bacc("TRN2", num_devices=NCORES, debug=False)
    xc_d = nc.dram_tensor("xc", (128, L * COLS), dt.bfloat16,
                          kind="ExternalInput")
    wx_d = nc.dram_tensor("wx", (128, 512), dt.bfloat16, kind="ExternalInput")
    wh_d = nc.dram_tensor("wh", (128, 512), dt.bfloat16, kind="ExternalInput")
    if with_bias:
        wb_d = nc.dram_tensor("wb", (128, 512), dt.bfloat16,
                              kind="ExternalInput")
    out_d = nc.dram_tensor("out", (128, L * COLS), dt.bfloat16,
                           kind="ExternalOutput")

    with tile.TileContext(nc) as tc:
        with (
            tc.tile_pool(name="big", bufs=1) as big,
            tc.tile_pool(name="zp", bufs=2, space="PSUM") as zp,
        ):
            XR = big.tile([128, RX * COLS], dt.bfloat16, name="XR", tag="XR")
            HR = big.tile([128, RH * COLS], dt.bfloat16, name="HR", tag="HR")
            wxt = big.tile([128, 512], dt.bfloat16, name="wxt", tag="wxt")
            wht = big.tile([128, 512], dt.bfloat16, name="wht", tag="wht")
            nc.sync.dma_start(out=wxt[:, :], in_=wx_d.ap())
            nc.sync.dma_start(out=wht[:, :], in_=wh_d.ap())
            if with_bias:
                wbt = big.tile([128, 512], dt.bfloat16, name="wbt", tag="wbt")
                nc.sync.dma_start(out=wbt[:, :], in_=wb_d.ap())
                ones = big.tile([128, COLS], dt.bfloat16, name="ones",
                                tag="ones")
                nc.vector.memset(ones[:, :], 1.0)
            FS = [GB[g + 1] - GB[g] for g in range(NG)]
            S_t, c_t, Tc_t, t1_t, t2_t = {}, {}, {}, {}, {}
            for g in range(NG):
                f = FS[g]
                S_t[g] = big.tile([128, 4 * f], dt.float32, name=f"S{g}",
                                  tag=f"S{g}")
                c_t[g] = big.tile([128, f], dt.float32, name=f"c{g}",
                                  tag=f"c{g}")
                nc.vector.memset(c_t[g][:, :], 0.0)
                Tc_t[g] = big.tile([128, f], dt.float32, name=f"Tc{g}",
                                   tag=f"Tc{g}")
                t1_t[g] = big.tile([128, f], dt.float32, name=f"t1{g}",
                                   tag=f"t1{g}")
                t2_t[g] = big.tile([128, f], dt.float32, name=f"t2{g}",
                                   tag=f"t2{g}")

            def xdma(b):
                lo = (b % 3) * T * COLS
                nc.sync.dma_start(out=XR[:, lo:lo + T * COLS],
                                  in_=xc_d.ap()[:, b * T * COLS:
                                                (b + 1) * T * COLS])

            # small first chunk so step 0 starts before the bulk lands
            nc.sync.dma_start(out=XR[:, 0:2 * COLS],
                              in_=xc_d.ap()[:, 0:2 * COLS])
            nc.sync.dma_start(out=XR[:, 2 * COLS:T * COLS],
                              in_=xc_d.ap()[:, 2 * COLS:T * COLS])
            for b in range(1, min(3, NB)):
                xdma(b)

            # Software-pipelined emission over virtual slots v = 3t + g.
            # Per slot: matmuls+gate-sigmoid for (g,t), then the cell-update
            # chain and tanh/h for the PREVIOUS slot, so the activation queue
            # alternates [A1_v, A2_{v-1}] and each group's serial chain hides
            # behind the other two groups' engine work.
            zt = {}

            def emit_mm_a1(v):
                g, t = v % NG, v // NG
                f, a = FS[g], GB[g]
                z = zp.tile([128, 4 * f], dt.float32, name=f"z{g}",
                            tag=f"z{g}")
                zt[v] = z
                xs = (t % RX) * COLS
                # start=True only on the first matmul into the psum tile:
                # start marks the whole 2KB zero-region pending-zero, so
                # later writes overwrite-as-zero once and then accumulate.
                rhs = XR[:, xs + a:xs + a + f]
                for q in range(4):
                    last = (t == 0) and not with_bias and q == 3
                    nc.tensor.matmul(z[:, q * f:(q + 1) * f],
                                     wxt[:, q * 128:(q + 1) * 128],
                                     rhs, start=(q == 0), stop=last)
                if with_bias:
                    for q in range(4):
                        nc.tensor.matmul(z[:, q * f:(q + 1) * f],
                                         wbt[:, q * 128:(q + 1) * 128],
                                         ones[:, a:a + f], start=False,
                                         stop=(t == 0) and q == 3)
                if t > 0:
                    hp = ((t - 1) % RH) * COLS
                    rhs = HR[:, hp + a:hp + a + f]
                    for q in range(4):
                        nc.tensor.matmul(z[:, q * f:(q + 1) * f],
                                         wht[:, q * 128:(q + 1) * 128],
                                         rhs, start=False, stop=(q == 3))
                nc.scalar.activation(S_t[g][:, :], z[:, :], AF.Sigmoid)

            def emit_cell(v):
                g, t = v % NG, v // NG
                f, a = FS[g], GB[g]
                nc.gpsimd.tensor_tensor(t2_t[g][:, :], S_t[g][:, f:2 * f],
                                        c_t[g][:, :], OP.mult)
                nc.vector.scalar_tensor_tensor(
                    out=t1_t[g][:, :], in0=S_t[g][:, 2 * f:3 * f],
                    scalar=0.5, in1=S_t[g][:, 0:f],
                    op0=OP.subtract, op1=OP.mult)
                nc.vector.scalar_tensor_tensor(
                    out=c_t[g][:, :], in0=t1_t[g][:, :], scalar=2.0,
                    in1=t2_t[g][:, :], op0=OP.mult, op1=OP.add)
                nc.scalar.activation(Tc_t[g][:, :], c_t[g][:, :], AF.Tanh)
                hs = (t % RH) * COLS
                nc.gpsimd.tensor_tensor(HR[:, hs + a:hs + a + f],
                                        Tc_t[g][:, :],
                                        S_t[g][:, 3 * f:4 * f], OP.mult)
                del zt[v]
                # x prefetch / h writeback bookkeeping on group-2 boundaries
                if g == 0 and t % T == 1 and t // T >= 1 and t // T + 2 < NB:
                    xdma(t // T + 2)
                if g == NG - 1 and t % T == T - 1:
                    k = t // T
                    lo = (k % 2) * T * COLS
                    nc.sync.dma_start(
                        out=out_d.ap()[:, k * T * COLS:(k + 1) * T * COLS],
                        in_=HR[:, lo:lo + T * COLS])

            # Stagger the three groups' recurrence chains by ~1/3 period:
            # dummy pool ops give group g's first gate-sigmoid a real
            # dependency on group g-1's, so the greedy scheduler settles
            # into the staggered (pipelined) steady state instead of the
            # slower lockstep one. The dummy writes one column of S which
            # the group's own A1 fully overwrites.
            def stagger(v):
                g = v % NG
                if v in (1, 2):
                    nc.gpsimd.tensor_tensor(S_t[g][:, 0:1],
                                            S_t[g - 1][:, 0:1],
                                            S_t[g - 1][:, 0:1], OP.mult)

            for v in range(NG * L):
                stagger(v)
                emit_mm_a1(v)
                if v >= 1:
                    emit_cell(v - 1)
            emit_cell(NG * L - 1)

    nc.compile()
    return nc


def _get_nc(with_bias):
    key = ("v6", with_bias)
    if key not in _CACHE:
        _CACHE[key] = _build(with_bias)
    return _CACHE[key]


def _prep_w(Wk, Wr, b):
    """Blockdiag-packed lhsT weights [128, 4*128] bf16 for x- and h-parts,
    plus optional rank-1 bias lhsT. Gate order i,f,g,o; g pre-scaled by 2
    (tanh via sigmoid)."""
    Wk = np.asarray(Wk, np.float32)
    Wr = np.asarray(Wr, np.float32)
    b = np.asarray(b, np.float32)
    wx = np.zeros((128, 512), np.float32)
    wh = np.zeros((128, 512), np.float32)
    wb = np.zeros((128, 512), np.float32)
    for q in range(4):
        sc = 2.0 if q == 2 else 1.0
        Wq = Wk[:, q * 64:(q + 1) * 64] * sc
        Rq = Wr[:, q * 64:(q + 1) * 64] * sc
        wx[0:64, q * 128 + 0:q * 128 + 64] = Wq
        wx[64:128, q * 128 + 64:q * 128 + 128] = Wq
        wh[0:64, q * 128 + 0:q * 128 + 64] = Rq
        wh[64:128, q * 128 + 64:q * 128 + 128] = Rq
        bq = b[q * 64:(q + 1) * 64] * sc
        wb[0, q * 128 + 0:q * 128 + 64] = bq
        wb[0, q * 128 + 64:q * 128 + 128] = bq
    with_bias = bool(np.any(b != 0.0))
    return (wx.astype(_BF16), wh.astype(_BF16), wb.astype(_BF16), with_bias)


def _pack_x(ids_q, emb, rev):
    """ids_q [512, L] -> packed [128, L*COLS] bf16 (p = hdim + 64*(s>=256))."""
    x = emb[ids_q]                                   # [512, L, E] f32
    if rev:
        x = x[:, ::-1, :]
    xr = x.reshape(2, COLS, L, E).transpose(0, 3, 2, 1)   # [2, E, L, COLS]
    return np.ascontiguousarray(xr).reshape(128, L * COLS).astype(_BF16)


def _unpack_h(o, rev):
    """[128, L*COLS] -> h [512, L, 64] f32."""
    o = np.asarray(o, np.float32).reshape(2, 64, L, COLS)
    h = o.transpose(0, 3, 2, 1).reshape(NC_SEQ, L, 64)
    if rev:
        h = h[:, ::-1, :]
    return h


def _in_maps(ids, embed_table, Wk_f, Wr_f, b_f, Wk_b, Wr_b, b_b):
    ids2 = np.asarray(ids).reshape(NSEQ, L)
    emb = np.asarray(embed_table, dtype=np.float32)
    wx_f, wh_f, wb_f, bias_f = _prep_w(Wk_f, Wr_f, b_f)
    wx_b, wh_b, wb_b, bias_b = _prep_w(Wk_b, Wr_b, b_b)
    with_bias = bias_f or bias_b
    in_maps = []
    for m in range(NCORES):
        rev = m >= NQ
        q = m % NQ
        ids_q = ids2[q * NC_SEQ:(q + 1) * NC_SEQ]
        im = {"xc": _pack_x(ids_q, emb, rev),
              "wx": wx_b if rev else wx_f,
              "wh": wh_b if rev else wh_f}
        if with_bias:
            im["wb"] = wb_b if rev else wb_f
        in_maps.append(im)
    return in_maps, with_bias


def kernel(ids, embed_table, Wk_f, Wr_f, b_f, Wk_b, Wr_b, b_b):
    from concourse import bass_utils

    in_maps, with_bias = _in_maps(ids, embed_table, Wk_f, Wr_f, b_f,
                                  Wk_b, Wr_b, b_b)
    nc = _get_nc(with_bias)
    res = bass_utils.run_bass_kernel_spmd(nc, in_maps,
                                          core_ids=list(range(NCORES)))

    out = np.empty((NSEQ, L, 2 * H), dtype=np.float32)
    for m in range(NCORES):
        rev = m >= NQ
        q = m % NQ
        h = _unpack_h(res.results[m]["out"], rev)
        sl = slice(q * NC_SEQ, (q + 1) * NC_SEQ)
        if rev:
            out[sl, :, H:2 * H] = h
        else:
            out[sl, :, 0:H] = h
    return out.reshape(B, S, L, 2 * H)


# revision 11
# speedup vs baseline: 1.1165x; 1.0022x over previous
"""Bidirectional LSTM over embedded event ids — Trainium2 Bass kernel.

Problem shapes (hardcoded): ids [32,64,256] int32, embed [6000,64],
per-direction LSTM E=H=64, output [32,64,256,128] f32.

Strategy: direction-parallel + data-parallel. Cores 0-3 run the forward
LSTM on sequence quarters 0-3 (512 seqs each); cores 4-7 run the backward
LSTM on the same quarters (host pre-reverses time). Per core, the 512
sequences are packed two-per-partition-lane: partition p = hdim + 64*(s
>= 256), free column j = s % 256. All per-step tensors are [128, 256]
with full partition utilization.

The 256 free columns split into G=3 interleaved groups (86/85/85) whose
recurrence chains overlap on the engines — the serial chain (matmul ->
sigmoid -> cell update -> tanh -> h-mul) of one group hides behind the
activation work of the other two.

Per group per step:
  - 8 matmuls (4 gates x {x-part, h-part}), bf16, accumulate in PSUM
    Z[128, 4f]; weights are [128,128] blockdiag(Wq, Wq) so both
    sequence halves share one matmul. g-gate weights pre-scaled by 2.
  - one Sigmoid over all 4 gates: S = sigma(Z)   (tanh(zg) = 2*sig(2 zg)-1)
  - t2 = S_f * c                     (tensor_tensor, Pool)
  - t1 = (S_g - 0.5) * S_i           (scalar_tensor_tensor, DVE)
  - c  = 2*t1 + t2                   (scalar_tensor_tensor, DVE)
  - Tc = tanh(c)                     (Act; same act table as sigmoid)
  - h  = Tc * S_o -> bf16 h-ring     (tensor_tensor, Pool — keeping the
    late-phase multiply off the DVE queue avoids a period-doubling
    oscillation between the three group pipelines)

x is streamed from HBM in T-step blocks into a 3T-slot bf16 ring; h is
written to a 2T-slot bf16 ring that doubles as matmul rhs source and
output staging (one batched DMA per T steps each way).
"""

import numpy as np
import ml_dtypes

B, S, L, E, H, V = 32, 64, 256, 64, 64, 6000
NCORES = 8
NSEQ = B * S               # 2048
NC_SEQ = 512               # sequences per core (one direction)
NQ = NSEQ // NC_SEQ        # 4 sequence quarters
COLS = NC_SEQ // 2         # 256 free columns (2 seqs per partition lane)
GB = [0, 86, 171, 256]     # group boundaries over free columns
NG = 3
T = 16                     # DMA block (timesteps)
RX = 3 * T                 # x ring slots
RH = 2 * T                 # h ring slots
NB = L // T

_CACHE = {}
_BF16 = ml_dtypes.bfloat16


def _build(with_bias):
    import concourse.bacc as bacc
    import concourse.tile as tile
    from concourse import mybir

    dt = mybir.dt
    AF = mybir.ActivationFunctionType
    OP = mybir.AluOpType

    nc = bacc.Bacc("TRN2", num_devices=NCORES, debug=False)
    xc_d = nc.dram_tensor("xc", (128, L * COLS), dt.bfloat16,
                          kind="ExternalInput")
    wx_d = nc.dram_tensor("wx", (128, 512), dt.bfloat16, kind="ExternalInput")
    wh_d = nc.dram_tensor("wh", (128, 512), dt.bfloat16, kind="ExternalInput")
    if with_bias:
        wb_d = nc.dram_tensor("wb", (128, 512), dt.bfloat16,
                              kind="ExternalInput")
    out_d = nc.dram_tensor("out", (128, L * COLS), dt.bfloat16,
                           kind="ExternalOutput")

    with tile.TileContext(nc) as tc:
        with (
            tc.tile_pool(name="big", bufs=1) as big,
            tc.tile_pool(name="zp", bufs=2, space="PSUM") as zp,
        ):
            XR = big.tile([128, RX * COLS], dt.bfloat16, name="XR", tag="XR")
            HR = big.tile([128, RH * COLS], dt.bfloat16, name="HR", tag="HR")
            wxt = big.tile([128, 512], dt.bfloat16, name="wxt", tag="wxt")
            wht = big.tile([128, 512], dt.bfloat16, name="wht", tag="wht")
            nc.sync.dma_start(out=wxt[:, :], in_=wx_d.ap())
            nc.sync.dma_start(out=wht[:, :], in_=wh_d.ap())
            if with_bias:
                wbt = big.tile([128, 512], dt.bfloat16, name="wbt", tag="wbt")
                nc.sync.dma_start(out=wbt[:, :], in_=wb_d.ap())
                ones = big.tile([128, COLS], dt.bfloat16, name="ones",
                                tag="ones")
                nc.vector.memset(ones[:, :], 1.0)
            FS = [GB[g + 1] - GB[g] for g in range(NG)]
            S_t, c_t, Tc_t, t1_t, t2_t = {}, {}, {}, {}, {}
            for g in range(NG):
                f = FS[g]
                S_t[g] = big.tile([128, 4 * f], dt.float32, name=f"S{g}",
                                  tag=f"S{g}")
                c_t[g] = big.tile([128, f], dt.float32, name=f"c{g}",
                                  tag=f"c{g}")
                nc.vector.memset(c_t[g][:, :], 0.0)
                Tc_t[g] = big.tile([128, f], dt.float32, name=f"Tc{g}",
                                   tag=f"Tc{g}")
                t1_t[g] = big.tile([128, f], dt.float32, name=f"t1{g}",
                                   tag=f"t1{g}")
                t2_t[g] = big.tile([128, f], dt.float32, name=f"t2{g}",
                                   tag=f"t2{g}")

            def xdma(b):
                lo = (b % 3) * T * COLS
                nc.sync.dma_start(out=XR[:, lo:lo + T * COLS],
                                  in_=xc_d.ap()[:, b * T * COLS:
                                                (b + 1) * T * COLS])

            # small first chunk so step 0 starts before the bulk lands
            nc.sync.dma_start(out=XR[:, 0:2 * COLS],
                              in_=xc_d.ap()[:, 0:2 * COLS])
            nc.sync.dma_start(out=XR[:, 2 * COLS:T * COLS],
                              in_=xc_d.ap()[:, 2 * COLS:T * COLS])
            for b in range(1, min(3, NB)):
                xdma(b)

            # Software-pipelined emission over virtual slots v = 3t + g.
            # Per slot: matmuls+gate-sigmoid for (g,t), then the cell-update
            # chain and tanh/h for the PREVIOUS slot, so the activation queue
            # alternates [A1_v, A2_{v-1}] and each group's serial chain hides
            # behind the other two groups' engine work.
            zt = {}

            def emit_mm_a1(v):
                g, t = v % NG, v // NG
                f, a = FS[g], GB[g]
                z = zp.tile([128, 4 * f], dt.float32, name=f"z{g}",
                            tag=f"z{g}")
                zt[v] = z
                xs = (t % RX) * COLS
                # start=True only on the first matmul into the psum tile:
                # start marks the whole 2KB zero-region pending-zero, so
                # later writes overwrite-as-zero once and then accumulate.
                rhs = XR[:, xs + a:xs + a + f]
                for q in range(4):
                    last = (t == 0) and not with_bias and q == 3
                    nc.tensor.matmul(z[:, q * f:(q + 1) * f],
                                     wxt[:, q * 128:(q + 1) * 128],
                                     rhs, start=(q == 0), stop=last)
                if with_bias:
                    for q in range(4):
                        nc.tensor.matmul(z[:, q * f:(q + 1) * f],
                                         wbt[:, q * 128:(q + 1) * 128],
                                         ones[:, a:a + f], start=False,
                                         stop=(t == 0) and q == 3)
                if t > 0:
                    hp = ((t - 1) % RH) * COLS
                    rhs = HR[:, hp + a:hp + a + f]
                    for q in range(4):
                        nc.tensor.matmul(z[:, q * f:(q + 1) * f],
                                         wht[:, q * 128:(q + 1) * 128],
                                         rhs, start=False, stop=(q == 3))
                nc.scalar.activation(S_t[g][:, :], z[:, :], AF.Sigmoid)

            def emit_cell(v):
                g, t = v % NG, v // NG
                f, a = FS[g], GB[g]
                nc.gpsimd.tensor_tensor(t2_t[g][:, :], S_t[g][:, f:2 * f],
                                        c_t[g][:, :], OP.mult)
                nc.vector.scalar_tensor_tensor(
                    out=t1_t[g][:, :], in0=S_t[g][:, 2 * f:3 * f],
                    scalar=0.5, in1=S_t[g][:, 0:f],
                    op0=OP.subtract, op1=OP.mult)
                nc.vector.scalar_tensor_tensor(
                    out=c_t[g][:, :], in0=t1_t[g][:, :], scalar=2.0,
                    in1=t2_t[g][:, :], op0=OP.mult, op1=OP.add)
                nc.scalar.activation(Tc_t[g][:, :], c_t[g][:, :], AF.Tanh)
                hs = (t % RH) * COLS
                nc.gpsimd.tensor_tensor(HR[:, hs + a:hs + a + f],
                                        Tc_t[g][:, :],
                                        S_t[g][:, 3 * f:4 * f], OP.mult)
                del zt[v]
                # x prefetch / h writeback bookkeeping on group-2 boundaries
                if g == 0 and t % T == 1 and t // T >= 1 and t // T + 2 < NB:
                    xdma(t // T + 2)
                if g == NG - 1 and t == L - 1 - T // 2:
                    # flush the final block's first half early so the tail
                    # DMA after the last step is half as long
                    k = NB - 1
                    lo = (k % 2) * T * COLS
                    nc.sync.dma_start(
                        out=out_d.ap()[:, k * T * COLS:
                                       (k * T + T // 2) * COLS],
                        in_=HR[:, lo:lo + (T // 2) * COLS])
                if g == NG - 1 and t % T == T - 1:
                    k = t // T
                    lo = (k % 2) * T * COLS
                    if k == NB - 1:
                        lo += (T // 2) * COLS
                        nc.sync.dma_start(
                            out=out_d.ap()[:, (k * T + T // 2) * COLS:
                                           (k + 1) * T * COLS],
                            in_=HR[:, lo:lo + (T // 2) * COLS])
                    else:
                        nc.sync.dma_start(
                            out=out_d.ap()[:, k * T * COLS:
                                           (k + 1) * T * COLS],
                            in_=HR[:, lo:lo + T * COLS])

            # Stagger the three groups' recurrence chains by ~1/3 period:
            # dummy pool ops give group g's first gate-sigmoid a real
            # dependency on group g-1's, so the greedy scheduler settles
            # into the staggered (pipelined) steady state instead of the
            # slower lockstep one. The dummy writes one column of S which
            # the group's own A1 fully overwrites.
            def stagger(v):
                g = v % NG
                if v in (1, 2):
                    nc.gpsimd.tensor_tensor(S_t[g][:, 0:1],
                                            S_t[g - 1][:, 0:1],
                                            S_t[g - 1][:, 0:1], OP.mult)

            for v in range(NG * L):
                stagger(v)
                emit_mm_a1(v)
                if v >= 1:
                    emit_cell(v - 1)
            emit_cell(NG * L - 1)

    nc.compile()
    return nc


def _get_nc(with_bias):
    key = ("v6", with_bias)
    if key not in _CACHE:
        _CACHE[key] = _build(with_bias)
    return _CACHE[key]


def _prep_w(Wk, Wr, b):
    """Blockdiag-packed lhsT weights [128, 4*128] bf16 for x- and h-parts,
    plus optional rank-1 bias lhsT. Gate order i,f,g,o; g pre-scaled by 2
    (tanh via sigmoid)."""
    Wk = np.asarray(Wk, np.float32)
    Wr = np.asarray(Wr, np.float32)
    b = np.asarray(b, np.float32)
    wx = np.zeros((128, 512), np.float32)
    wh = np.zeros((128, 512), np.float32)
    wb = np.zeros((128, 512), np.float32)
    for q in range(4):
        sc = 2.0 if q == 2 else 1.0
        Wq = Wk[:, q * 64:(q + 1) * 64] * sc
        Rq = Wr[:, q * 64:(q + 1) * 64] * sc
        wx[0:64, q * 128 + 0:q * 128 + 64] = Wq
        wx[64:128, q * 128 + 64:q * 128 + 128] = Wq
        wh[0:64, q * 128 + 0:q * 128 + 64] = Rq
        wh[64:128, q * 128 + 64:q * 128 + 128] = Rq
        bq = b[q * 64:(q + 1) * 64] * sc
        wb[0, q * 128 + 0:q * 128 + 64] = bq
        wb[0, q * 128 + 64:q * 128 + 128] = bq
    with_bias = bool(np.any(b != 0.0))
    return (wx.astype(_BF16), wh.astype(_BF16), wb.astype(_BF16), with_bias)


def _pack_x(ids_q, emb, rev):
    """ids_q [512, L] -> packed [128, L*COLS] bf16 (p = hdim + 64*(s>=256))."""
    x = emb[ids_q]                                   # [512, L, E] f32
    if rev:
        x = x[:, ::-1, :]
    xr = x.reshape(2, COLS, L, E).transpose(0, 3, 2, 1)   # [2, E, L, COLS]
    return np.ascontiguousarray(xr).reshape(128, L * COLS).astype(_BF16)


def _unpack_h(o, rev):
    """[128, L*COLS] -> h [512, L, 64] f32."""
    o = np.asarray(o, np.float32).reshape(2, 64, L, COLS)
    h = o.transpose(0, 3, 2, 1).reshape(NC_SEQ, L, 64)
    if rev:
        h = h[:, ::-1, :]
    return h


def _in_maps(ids, embed_table, Wk_f, Wr_f, b_f, Wk_b, Wr_b, b_b):
    ids2 = np.asarray(ids).reshape(NSEQ, L)
    emb = np.asarray(embed_table, dtype=np.float32)
    wx_f, wh_f, wb_f, bias_f = _prep_w(Wk_f, Wr_f, b_f)
    wx_b, wh_b, wb_b, bias_b = _prep_w(Wk_b, Wr_b, b_b)
    with_bias = bias_f or bias_b
    in_maps = []
    for m in range(NCORES):
        rev = m >= NQ
        q = m % NQ
        ids_q = ids2[q * NC_SEQ:(q + 1) * NC_SEQ]
        im = {"xc": _pack_x(ids_q, emb, rev),
              "wx": wx_b if rev else wx_f,
              "wh": wh_b if rev else wh_f}
        if with_bias:
            im["wb"] = wb_b if rev else wb_f
        in_maps.append(im)
    return in_maps, with_bias


def kernel(ids, embed_table, Wk_f, Wr_f, b_f, Wk_b, Wr_b, b_b):
    from concourse import bass_utils

    in_maps, with_bias = _in_maps(ids, embed_table, Wk_f, Wr_f, b_f,
                                  Wk_b, Wr_b, b_b)
    nc = _get_nc(with_bias)
    res = bass_utils.run_bass_kernel_spmd(nc, in_maps,
                                          core_ids=list(range(NCORES)))

    out = np.empty((NSEQ, L, 2 * H), dtype=np.float32)
    for m in range(NCORES):
        rev = m >= NQ
        q = m % NQ
        h = _unpack_h(res.results[m]["out"], rev)
        sl = slice(q * NC_SEQ, (q + 1) * NC_SEQ)
        if rev:
            out[sl, :, H:2 * H] = h
        else:
            out[sl, :, 0:H] = h
    return out.reshape(B, S, L, 2 * H)


# revision 12
# speedup vs baseline: 1.1177x; 1.0011x over previous
"""Bidirectional LSTM over embedded event ids — Trainium2 Bass kernel.

Problem shapes (hardcoded): ids [32,64,256] int32, embed [6000,64],
per-direction LSTM E=H=64, output [32,64,256,128] f32.

Strategy: direction-parallel + data-parallel. Cores 0-3 run the forward
LSTM on sequence quarters 0-3 (512 seqs each); cores 4-7 run the backward
LSTM on the same quarters (host pre-reverses time). Per core, the 512
sequences are packed two-per-partition-lane: partition p = hdim + 64*(s
>= 256), free column j = s % 256. All per-step tensors are [128, 256]
with full partition utilization.

The 256 free columns split into G=3 interleaved groups (86/85/85) whose
recurrence chains overlap on the engines — the serial chain (matmul ->
sigmoid -> cell update -> tanh -> h-mul) of one group hides behind the
activation work of the other two.

Per group per step:
  - 8 matmuls (4 gates x {x-part, h-part}), bf16, accumulate in PSUM
    Z[128, 4f]; weights are [128,128] blockdiag(Wq, Wq) so both
    sequence halves share one matmul. g-gate weights pre-scaled by 2.
  - one Sigmoid over all 4 gates: S = sigma(Z)   (tanh(zg) = 2*sig(2 zg)-1)
  - t2 = S_f * c                     (tensor_tensor, Pool)
  - t1 = (S_g - 0.5) * S_i           (scalar_tensor_tensor, DVE)
  - c  = 2*t1 + t2                   (scalar_tensor_tensor, DVE)
  - Tc = tanh(c)                     (Act; same act table as sigmoid)
  - h  = Tc * S_o -> bf16 h-ring     (tensor_tensor, Pool — keeping the
    late-phase multiply off the DVE queue avoids a period-doubling
    oscillation between the three group pipelines)

x is streamed from HBM in T-step blocks into a 3T-slot bf16 ring; h is
written to a 2T-slot bf16 ring that doubles as matmul rhs source and
output staging (one batched DMA per T steps each way).
"""

import numpy as np
import ml_dtypes

B, S, L, E, H, V = 32, 64, 256, 64, 64, 6000
NCORES = 8
NSEQ = B * S               # 2048
NC_SEQ = 512               # sequences per core (one direction)
NQ = NSEQ // NC_SEQ        # 4 sequence quarters
COLS = NC_SEQ // 2         # 256 free columns (2 seqs per partition lane)
GB = [0, 86, 171, 256]     # group boundaries over free columns
NG = 3
T = 16                     # DMA block (timesteps)
RX = 3 * T                 # x ring slots
RH = 2 * T                 # h ring slots
NB = L // T

_CACHE = {}
_BF16 = ml_dtypes.bfloat16


def _build(with_bias):
    import concourse.bacc as bacc
    import concourse.tile as tile
    from concourse import mybir

    dt = mybir.dt
    AF = mybir.ActivationFunctionType
    OP = mybir.AluOpType

    nc = bacc.Bacc("TRN2", num_devices=NCORES, debug=False)
    xc_d = nc.dram_tensor("xc", (128, L * COLS), dt.bfloat16,
                          kind="ExternalInput")
    wx_d = nc.dram_tensor("wx", (128, 512), dt.bfloat16, kind="ExternalInput")
    wh_d = nc.dram_tensor("wh", (128, 512), dt.bfloat16, kind="ExternalInput")
    if with_bias:
        wb_d = nc.dram_tensor("wb", (128, 512), dt.bfloat16,
                              kind="ExternalInput")
    out_d = nc.dram_tensor("out", (128, L * COLS), dt.bfloat16,
                           kind="ExternalOutput")

    with tile.TileContext(nc) as tc:
        with (
            tc.tile_pool(name="big", bufs=1) as big,
            tc.tile_pool(name="zp", bufs=2, space="PSUM") as zp,
        ):
            XR = big.tile([128, RX * COLS], dt.bfloat16, name="XR", tag="XR")
            HR = big.tile([128, RH * COLS], dt.bfloat16, name="HR", tag="HR")
            wxt = big.tile([128, 512], dt.bfloat16, name="wxt", tag="wxt")
            wht = big.tile([128, 512], dt.bfloat16, name="wht", tag="wht")
            nc.sync.dma_start(out=wxt[:, :], in_=wx_d.ap())
            nc.sync.dma_start(out=wht[:, :], in_=wh_d.ap())
            if with_bias:
                wbt = big.tile([128, 512], dt.bfloat16, name="wbt", tag="wbt")
                nc.sync.dma_start(out=wbt[:, :], in_=wb_d.ap())
                ones = big.tile([128, COLS], dt.bfloat16, name="ones",
                                tag="ones")
                nc.vector.memset(ones[:, :], 1.0)
            FS = [GB[g + 1] - GB[g] for g in range(NG)]
            S_t, c_t, Tc_t, t1_t, t2_t = {}, {}, {}, {}, {}
            for g in range(NG):
                f = FS[g]
                S_t[g] = big.tile([128, 4 * f], dt.float32, name=f"S{g}",
                                  tag=f"S{g}")
                c_t[g] = big.tile([128, f], dt.float32, name=f"c{g}",
                                  tag=f"c{g}")
                nc.vector.memset(c_t[g][:, :], 0.0)
                Tc_t[g] = big.tile([128, f], dt.float32, name=f"Tc{g}",
                                   tag=f"Tc{g}")
                t1_t[g] = big.tile([128, f], dt.float32, name=f"t1{g}",
                                   tag=f"t1{g}")
                t2_t[g] = big.tile([128, f], dt.float32, name=f"t2{g}",
                                   tag=f"t2{g}")

            def xdma(b):
                lo = (b % 3) * T * COLS
                nc.sync.dma_start(out=XR[:, lo:lo + T * COLS],
                                  in_=xc_d.ap()[:, b * T * COLS:
                                                (b + 1) * T * COLS])

            # small first chunk so step 0 starts before the bulk lands
            nc.sync.dma_start(out=XR[:, 0:2 * COLS],
                              in_=xc_d.ap()[:, 0:2 * COLS])
            nc.sync.dma_start(out=XR[:, 2 * COLS:T * COLS],
                              in_=xc_d.ap()[:, 2 * COLS:T * COLS])
            for b in range(1, min(3, NB)):
                xdma(b)

            # Software-pipelined emission over virtual slots v = 3t + g.
            # Per slot: matmuls+gate-sigmoid for (g,t), then the cell-update
            # chain and tanh/h for the PREVIOUS slot, so the activation queue
            # alternates [A1_v, A2_{v-1}] and each group's serial chain hides
            # behind the other two groups' engine work.
            zt = {}

            def emit_mm_a1(v):
                g, t = v % NG, v // NG
                f, a = FS[g], GB[g]
                z = zp.tile([128, 4 * f], dt.float32, name=f"z{g}",
                            tag=f"z{g}")
                zt[v] = z
                xs = (t % RX) * COLS
                # start=True only on the first matmul into the psum tile:
                # start marks the whole 2KB zero-region pending-zero, so
                # later writes overwrite-as-zero once and then accumulate.
                rhs = XR[:, xs + a:xs + a + f]
                for q in range(4):
                    last = (t == 0) and not with_bias and q == 3
                    nc.tensor.matmul(z[:, q * f:(q + 1) * f],
                                     wxt[:, q * 128:(q + 1) * 128],
                                     rhs, start=(q == 0), stop=last)
                if with_bias:
                    for q in range(4):
                        nc.tensor.matmul(z[:, q * f:(q + 1) * f],
                                         wbt[:, q * 128:(q + 1) * 128],
                                         ones[:, a:a + f], start=False,
                                         stop=(t == 0) and q == 3)
                if t > 0:
                    hp = ((t - 1) % RH) * COLS
                    rhs = HR[:, hp + a:hp + a + f]
                    for q in range(4):
                        nc.tensor.matmul(z[:, q * f:(q + 1) * f],
                                         wht[:, q * 128:(q + 1) * 128],
                                         rhs, start=False, stop=(q == 3))
                nc.scalar.activation(S_t[g][:, :], z[:, :], AF.Sigmoid)

            def emit_cell(v):
                g, t = v % NG, v // NG
                f, a = FS[g], GB[g]
                nc.gpsimd.tensor_tensor(t2_t[g][:, :], S_t[g][:, f:2 * f],
                                        c_t[g][:, :], OP.mult)
                nc.vector.scalar_tensor_tensor(
                    out=t1_t[g][:, :], in0=S_t[g][:, 2 * f:3 * f],
                    scalar=0.5, in1=S_t[g][:, 0:f],
                    op0=OP.subtract, op1=OP.mult)
                nc.vector.scalar_tensor_tensor(
                    out=c_t[g][:, :], in0=t1_t[g][:, :], scalar=2.0,
                    in1=t2_t[g][:, :], op0=OP.mult, op1=OP.add)
                nc.scalar.activation(Tc_t[g][:, :], c_t[g][:, :], AF.Tanh)
                hs = (t % RH) * COLS
                nc.gpsimd.tensor_tensor(HR[:, hs + a:hs + a + f],
                                        Tc_t[g][:, :],
                                        S_t[g][:, 3 * f:4 * f], OP.mult)
                del zt[v]
                # x prefetch / h writeback bookkeeping on group-2 boundaries
                if g == 0 and t % T == 1 and t // T >= 1 and t // T + 2 < NB:
                    xdma(t // T + 2)
                if g == NG - 1 and t >= L - T:
                    # flush the final block incrementally (half, quarter,
                    # quarter) so the tail DMA after the last step is short
                    k = NB - 1
                    lo = (k % 2) * T * COLS
                    done = t - (L - T)  # slots complete in the last block
                    marks = {T // 2 - 1: (0, T // 2),
                             3 * T // 4 - 1: (T // 2, 3 * T // 4),
                             T - 1: (3 * T // 4, T)}
                    if done in marks:
                        s0, s1 = marks[done]
                        nc.sync.dma_start(
                            out=out_d.ap()[:, (k * T + s0) * COLS:
                                           (k * T + s1) * COLS],
                            in_=HR[:, lo + s0 * COLS:lo + s1 * COLS])
                elif g == NG - 1 and t % T == T - 1:
                    k = t // T
                    lo = (k % 2) * T * COLS
                    nc.sync.dma_start(
                        out=out_d.ap()[:, k * T * COLS:
                                       (k + 1) * T * COLS],
                        in_=HR[:, lo:lo + T * COLS])

            # Stagger the three groups' recurrence chains by ~1/3 period:
            # dummy pool ops give group g's first gate-sigmoid a real
            # dependency on group g-1's, so the greedy scheduler settles
            # into the staggered (pipelined) steady state instead of the
            # slower lockstep one. The dummy writes one column of S which
            # the group's own A1 fully overwrites.
            def stagger(v):
                g = v % NG
                if v in (1, 2):
                    nc.gpsimd.tensor_tensor(S_t[g][:, 0:1],
                                            S_t[g - 1][:, 0:1],
                                            S_t[g - 1][:, 0:1], OP.mult)

            for v in range(NG * L):
                stagger(v)
                emit_mm_a1(v)
                if v >= 1:
                    emit_cell(v - 1)
            emit_cell(NG * L - 1)

    nc.compile()
    return nc


def _get_nc(with_bias):
    key = ("v6", with_bias)
    if key not in _CACHE:
        _CACHE[key] = _build(with_bias)
    return _CACHE[key]


def _prep_w(Wk, Wr, b):
    """Blockdiag-packed lhsT weights [128, 4*128] bf16 for x- and h-parts,
    plus optional rank-1 bias lhsT. Gate order i,f,g,o; g pre-scaled by 2
    (tanh via sigmoid)."""
    Wk = np.asarray(Wk, np.float32)
    Wr = np.asarray(Wr, np.float32)
    b = np.asarray(b, np.float32)
    wx = np.zeros((128, 512), np.float32)
    wh = np.zeros((128, 512), np.float32)
    wb = np.zeros((128, 512), np.float32)
    for q in range(4):
        sc = 2.0 if q == 2 else 1.0
        Wq = Wk[:, q * 64:(q + 1) * 64] * sc
        Rq = Wr[:, q * 64:(q + 1) * 64] * sc
        wx[0:64, q * 128 + 0:q * 128 + 64] = Wq
        wx[64:128, q * 128 + 64:q * 128 + 128] = Wq
        wh[0:64, q * 128 + 0:q * 128 + 64] = Rq
        wh[64:128, q * 128 + 64:q * 128 + 128] = Rq
        bq = b[q * 64:(q + 1) * 64] * sc
        wb[0, q * 128 + 0:q * 128 + 64] = bq
        wb[0, q * 128 + 64:q * 128 + 128] = bq
    with_bias = bool(np.any(b != 0.0))
    return (wx.astype(_BF16), wh.astype(_BF16), wb.astype(_BF16), with_bias)


def _pack_x(ids_q, emb, rev):
    """ids_q [512, L] -> packed [128, L*COLS] bf16 (p = hdim + 64*(s>=256))."""
    x = emb[ids_q]                                   # [512, L, E] f32
    if rev:
        x = x[:, ::-1, :]
    xr = x.reshape(2, COLS, L, E).transpose(0, 3, 2, 1)   # [2, E, L, COLS]
    return np.ascontiguousarray(xr).reshape(128, L * COLS).astype(_BF16)


def _unpack_h(o, rev):
    """[128, L*COLS] -> h [512, L, 64] f32."""
    o = np.asarray(o, np.float32).reshape(2, 64, L, COLS)
    h = o.transpose(0, 3, 2, 1).reshape(NC_SEQ, L, 64)
    if rev:
        h = h[:, ::-1, :]
    return h


def _in_maps(ids, embed_table, Wk_f, Wr_f, b_f, Wk_b, Wr_b, b_b):
    ids2 = np.asarray(ids).reshape(NSEQ, L)
    emb = np.asarray(embed_table, dtype=np.float32)
    wx_f, wh_f, wb_f, bias_f = _prep_w(Wk_f, Wr_f, b_f)
    wx_b, wh_b, wb_b, bias_b = _prep_w(Wk_b, Wr_b, b_b)
    with_bias = bias_f or bias_b
    in_maps = []
    for m in range(NCORES):
        rev = m >= NQ
        q = m % NQ
        ids_q = ids2[q * NC_SEQ:(q + 1) * NC_SEQ]
        im = {"xc": _pack_x(ids_q, emb, rev),
              "wx": wx_b if rev else wx_f,
              "wh": wh_b if rev else wh_f}
        if with_bias:
            im["wb"] = wb_b if rev else wb_f
        in_maps.append(im)
    return in_maps, with_bias


def kernel(ids, embed_table, Wk_f, Wr_f, b_f, Wk_b, Wr_b, b_b):
    from concourse import bass_utils

    in_maps, with_bias = _in_maps(ids, embed_table, Wk_f, Wr_f, b_f,
                                  Wk_b, Wr_b, b_b)
    nc = _get_nc(with_bias)
    res = bass_utils.run_bass_kernel_spmd(nc, in_maps,
                                          core_ids=list(range(NCORES)))

    out = np.empty((NSEQ, L, 2 * H), dtype=np.float32)
    for m in range(NCORES):
        rev = m >= NQ
        q = m % NQ
        h = _unpack_h(res.results[m]["out"], rev)
        sl = slice(q * NC_SEQ, (q + 1) * NC_SEQ)
        if rev:
            out[sl, :, H:2 * H] = h
        else:
            out[sl, :, 0:H] = h
    return out.reshape(B, S, L, 2 * H)


# revision 13
# speedup vs baseline: 1.1181x; 1.0004x over previous
"""Bidirectional LSTM over embedded event ids — Trainium2 Bass kernel.

Problem shapes (hardcoded): ids [32,64,256] int32, embed [6000,64],
per-direction LSTM E=H=64, output [32,64,256,128] f32.

Strategy: direction-parallel + data-parallel. Cores 0-3 run the forward
LSTM on sequence quarters 0-3 (512 seqs each); cores 4-7 run the backward
LSTM on the same quarters (host pre-reverses time). Per core, the 512
sequences are packed two-per-partition-lane: partition p = hdim + 64*(s
>= 256), free column j = s % 256. All per-step tensors are [128, 256]
with full partition utilization.

The 256 free columns split into G=3 interleaved groups (86/85/85) whose
recurrence chains overlap on the engines — the serial chain (matmul ->
sigmoid -> cell update -> tanh -> h-mul) of one group hides behind the
activation work of the other two.

Per group per step:
  - 8 matmuls (4 gates x {x-part, h-part}), bf16, accumulate in PSUM
    Z[128, 4f]; weights are [128,128] blockdiag(Wq, Wq) so both
    sequence halves share one matmul. g-gate weights pre-scaled by 2.
  - one Sigmoid over all 4 gates: S = sigma(Z)   (tanh(zg) = 2*sig(2 zg)-1)
  - t2 = S_f * c                     (tensor_tensor, Pool)
  - t1 = (S_g - 0.5) * S_i           (scalar_tensor_tensor, DVE)
  - c  = 2*t1 + t2                   (scalar_tensor_tensor, DVE)
  - Tc = tanh(c)                     (Act; same act table as sigmoid)
  - h  = Tc * S_o -> bf16 h-ring     (tensor_tensor, Pool — keeping the
    late-phase multiply off the DVE queue avoids a period-doubling
    oscillation between the three group pipelines)

x is streamed from HBM in T-step blocks into a 3T-slot bf16 ring; h is
written to a 2T-slot bf16 ring that doubles as matmul rhs source and
output staging (one batched DMA per T steps each way).
"""

import numpy as np
import ml_dtypes

B, S, L, E, H, V = 32, 64, 256, 64, 64, 6000
NCORES = 8
NSEQ = B * S               # 2048
NC_SEQ = 512               # sequences per core (one direction)
NQ = NSEQ // NC_SEQ        # 4 sequence quarters
COLS = NC_SEQ // 2         # 256 free columns (2 seqs per partition lane)
GB = [0, 86, 171, 256]     # group boundaries over free columns
NG = 3
T = 16                     # DMA block (timesteps)
RX = 3 * T                 # x ring slots
RH = 2 * T                 # h ring slots
NB = L // T

_CACHE = {}
_BF16 = ml_dtypes.bfloat16


def _build(with_bias):
    import concourse.bacc as bacc
    import concourse.tile as tile
    from concourse import mybir

    dt = mybir.dt
    AF = mybir.ActivationFunctionType
    OP = mybir.AluOpType

    nc = bacc.Bacc("TRN2", num_devices=NCORES, debug=False)
    xc_d = nc.dram_tensor("xc", (128, L * COLS), dt.bfloat16,
                          kind="ExternalInput")
    w_d = nc.dram_tensor("w", (128, 1024), dt.bfloat16, kind="ExternalInput")
    if with_bias:
        wb_d = nc.dram_tensor("wb", (128, 512), dt.bfloat16,
                              kind="ExternalInput")
    out_d = nc.dram_tensor("out", (128, L * COLS), dt.bfloat16,
                           kind="ExternalOutput")

    with tile.TileContext(nc) as tc:
        with (
            tc.tile_pool(name="big", bufs=1) as big,
            tc.tile_pool(name="zp", bufs=2, space="PSUM") as zp,
        ):
            XR = big.tile([128, RX * COLS], dt.bfloat16, name="XR", tag="XR")
            HR = big.tile([128, RH * COLS], dt.bfloat16, name="HR", tag="HR")
            wt = big.tile([128, 1024], dt.bfloat16, name="wt", tag="wt")
            nc.sync.dma_start(out=wt[:, :], in_=w_d.ap())
            if with_bias:
                wbt = big.tile([128, 512], dt.bfloat16, name="wbt", tag="wbt")
                nc.sync.dma_start(out=wbt[:, :], in_=wb_d.ap())
                ones = big.tile([128, COLS], dt.bfloat16, name="ones",
                                tag="ones")
                nc.vector.memset(ones[:, :], 1.0)
            FS = [GB[g + 1] - GB[g] for g in range(NG)]
            S_t, c_t, Tc_t, t1_t, t2_t = {}, {}, {}, {}, {}
            for g in range(NG):
                f = FS[g]
                S_t[g] = big.tile([128, 4 * f], dt.float32, name=f"S{g}",
                                  tag=f"S{g}")
                c_t[g] = big.tile([128, f], dt.float32, name=f"c{g}",
                                  tag=f"c{g}")
                nc.vector.memset(c_t[g][:, :], 0.0)
                Tc_t[g] = big.tile([128, f], dt.float32, name=f"Tc{g}",
                                   tag=f"Tc{g}")
                t1_t[g] = big.tile([128, f], dt.float32, name=f"t1{g}",
                                   tag=f"t1{g}")
                t2_t[g] = big.tile([128, f], dt.float32, name=f"t2{g}",
                                   tag=f"t2{g}")

            def xdma(b):
                lo = (b % 3) * T * COLS
                nc.sync.dma_start(out=XR[:, lo:lo + T * COLS],
                                  in_=xc_d.ap()[:, b * T * COLS:
                                                (b + 1) * T * COLS])

            # small first chunk so step 0 starts before the bulk lands
            nc.sync.dma_start(out=XR[:, 0:COLS],
                              in_=xc_d.ap()[:, 0:COLS])
            nc.sync.dma_start(out=XR[:, COLS:T * COLS],
                              in_=xc_d.ap()[:, COLS:T * COLS])
            for b in range(1, min(3, NB)):
                xdma(b)

            # Software-pipelined emission over virtual slots v = 3t + g.
            # Per slot: matmuls+gate-sigmoid for (g,t), then the cell-update
            # chain and tanh/h for the PREVIOUS slot, so the activation queue
            # alternates [A1_v, A2_{v-1}] and each group's serial chain hides
            # behind the other two groups' engine work.
            zt = {}

            def emit_mm_a1(v):
                g, t = v % NG, v // NG
                f, a = FS[g], GB[g]
                z = zp.tile([128, 4 * f], dt.float32, name=f"z{g}",
                            tag=f"z{g}")
                zt[v] = z
                xs = (t % RX) * COLS
                # start=True only on the first matmul into the psum tile:
                # start marks the whole 2KB zero-region pending-zero, so
                # later writes overwrite-as-zero once and then accumulate.
                rhs = XR[:, xs + a:xs + a + f]
                for q in range(4):
                    last = (t == 0) and not with_bias and q == 3
                    nc.tensor.matmul(z[:, q * f:(q + 1) * f],
                                     wt[:, q * 128:(q + 1) * 128],
                                     rhs, start=(q == 0), stop=last)
                if with_bias:
                    for q in range(4):
                        nc.tensor.matmul(z[:, q * f:(q + 1) * f],
                                         wbt[:, q * 128:(q + 1) * 128],
                                         ones[:, a:a + f], start=False,
                                         stop=(t == 0) and q == 3)
                if t > 0:
                    hp = ((t - 1) % RH) * COLS
                    rhs = HR[:, hp + a:hp + a + f]
                    for q in range(4):
                        nc.tensor.matmul(z[:, q * f:(q + 1) * f],
                                         wt[:, 512 + q * 128:512 + (q + 1) * 128],
                                         rhs, start=False, stop=(q == 3))
                nc.scalar.activation(S_t[g][:, :], z[:, :], AF.Sigmoid)

            def emit_cell(v):
                g, t = v % NG, v // NG
                f, a = FS[g], GB[g]
                nc.gpsimd.tensor_tensor(t2_t[g][:, :], S_t[g][:, f:2 * f],
                                        c_t[g][:, :], OP.mult)
                nc.vector.scalar_tensor_tensor(
                    out=t1_t[g][:, :], in0=S_t[g][:, 2 * f:3 * f],
                    scalar=0.5, in1=S_t[g][:, 0:f],
                    op0=OP.subtract, op1=OP.mult)
                nc.vector.scalar_tensor_tensor(
                    out=c_t[g][:, :], in0=t1_t[g][:, :], scalar=2.0,
                    in1=t2_t[g][:, :], op0=OP.mult, op1=OP.add)
                nc.scalar.activation(Tc_t[g][:, :], c_t[g][:, :], AF.Tanh)
                hs = (t % RH) * COLS
                nc.gpsimd.tensor_tensor(HR[:, hs + a:hs + a + f],
                                        Tc_t[g][:, :],
                                        S_t[g][:, 3 * f:4 * f], OP.mult)
                del zt[v]
                # x prefetch / h writeback bookkeeping on group-2 boundaries
                if g == 0 and t % T == 1 and t // T >= 1 and t // T + 2 < NB:
                    xdma(t // T + 2)
                if g == NG - 1 and t >= L - T:
                    # flush the final block incrementally (half, quarter,
                    # quarter) so the tail DMA after the last step is short
                    k = NB - 1
                    lo = (k % 2) * T * COLS
                    done = t - (L - T)  # slots complete in the last block
                    marks = {T // 2 - 1: (0, T // 2),
                             3 * T // 4 - 1: (T // 2, 3 * T // 4),
                             T - 1: (3 * T // 4, T)}
                    if done in marks:
                        s0, s1 = marks[done]
                        nc.sync.dma_start(
                            out=out_d.ap()[:, (k * T + s0) * COLS:
                                           (k * T + s1) * COLS],
                            in_=HR[:, lo + s0 * COLS:lo + s1 * COLS])
                elif g == NG - 1 and t % T == T - 1:
                    k = t // T
                    lo = (k % 2) * T * COLS
                    nc.sync.dma_start(
                        out=out_d.ap()[:, k * T * COLS:
                                       (k + 1) * T * COLS],
                        in_=HR[:, lo:lo + T * COLS])

            # Stagger the three groups' recurrence chains by ~1/3 period:
            # dummy pool ops give group g's first gate-sigmoid a real
            # dependency on group g-1's, so the greedy scheduler settles
            # into the staggered (pipelined) steady state instead of the
            # slower lockstep one. The dummy writes one column of S which
            # the group's own A1 fully overwrites.
            def stagger(v):
                g = v % NG
                if v in (1, 2):
                    nc.gpsimd.tensor_tensor(S_t[g][:, 0:1],
                                            S_t[g - 1][:, 0:1],
                                            S_t[g - 1][:, 0:1], OP.mult)

            for v in range(NG * L):
                stagger(v)
                emit_mm_a1(v)
                if v >= 1:
                    emit_cell(v - 1)
            emit_cell(NG * L - 1)

    nc.compile()
    return nc


def _get_nc(with_bias):
    key = ("v6", with_bias)
    if key not in _CACHE:
        _CACHE[key] = _build(with_bias)
    return _CACHE[key]


def _prep_w(Wk, Wr, b):
    """Blockdiag-packed lhsT weights [128, 4*128] bf16 for x- and h-parts,
    plus optional rank-1 bias lhsT. Gate order i,f,g,o; g pre-scaled by 2
    (tanh via sigmoid)."""
    Wk = np.asarray(Wk, np.float32)
    Wr = np.asarray(Wr, np.float32)
    b = np.asarray(b, np.float32)
    wx = np.zeros((128, 512), np.float32)
    wh = np.zeros((128, 512), np.float32)
    wb = np.zeros((128, 512), np.float32)
    for q in range(4):
        sc = 2.0 if q == 2 else 1.0
        Wq = Wk[:, q * 64:(q + 1) * 64] * sc
        Rq = Wr[:, q * 64:(q + 1) * 64] * sc
        wx[0:64, q * 128 + 0:q * 128 + 64] = Wq
        wx[64:128, q * 128 + 64:q * 128 + 128] = Wq
        wh[0:64, q * 128 + 0:q * 128 + 64] = Rq
        wh[64:128, q * 128 + 64:q * 128 + 128] = Rq
        bq = b[q * 64:(q + 1) * 64] * sc
        wb[0, q * 128 + 0:q * 128 + 64] = bq
        wb[0, q * 128 + 64:q * 128 + 128] = bq
    with_bias = bool(np.any(b != 0.0))
    return (wx.astype(_BF16), wh.astype(_BF16), wb.astype(_BF16), with_bias)


def _pack_x(ids_q, emb, rev):
    """ids_q [512, L] -> packed [128, L*COLS] bf16 (p = hdim + 64*(s>=256))."""
    x = emb[ids_q]                                   # [512, L, E] f32
    if rev:
        x = x[:, ::-1, :]
    xr = x.reshape(2, COLS, L, E).transpose(0, 3, 2, 1)   # [2, E, L, COLS]
    return np.ascontiguousarray(xr).reshape(128, L * COLS).astype(_BF16)


def _unpack_h(o, rev):
    """[128, L*COLS] -> h [512, L, 64] f32."""
    o = np.asarray(o, np.float32).reshape(2, 64, L, COLS)
    h = o.transpose(0, 3, 2, 1).reshape(NC_SEQ, L, 64)
    if rev:
        h = h[:, ::-1, :]
    return h


def _in_maps(ids, embed_table, Wk_f, Wr_f, b_f, Wk_b, Wr_b, b_b):
    ids2 = np.asarray(ids).reshape(NSEQ, L)
    emb = np.asarray(embed_table, dtype=np.float32)
    wx_f, wh_f, wb_f, bias_f = _prep_w(Wk_f, Wr_f, b_f)
    wx_b, wh_b, wb_b, bias_b = _prep_w(Wk_b, Wr_b, b_b)
    with_bias = bias_f or bias_b
    in_maps = []
    for m in range(NCORES):
        rev = m >= NQ
        q = m % NQ
        ids_q = ids2[q * NC_SEQ:(q + 1) * NC_SEQ]
        im = {"xc": _pack_x(ids_q, emb, rev),
              "w": np.concatenate([wx_b, wh_b] if rev else [wx_f, wh_f],
                                  axis=1)}
        if with_bias:
            im["wb"] = wb_b if rev else wb_f
        in_maps.append(im)
    return in_maps, with_bias


def kernel(ids, embed_table, Wk_f, Wr_f, b_f, Wk_b, Wr_b, b_b):
    from concourse import bass_utils

    in_maps, with_bias = _in_maps(ids, embed_table, Wk_f, Wr_f, b_f,
                                  Wk_b, Wr_b, b_b)
    nc = _get_nc(with_bias)
    res = bass_utils.run_bass_kernel_spmd(nc, in_maps,
                                          core_ids=list(range(NCORES)))

    out = np.empty((NSEQ, L, 2 * H), dtype=np.float32)
    for m in range(NCORES):
        rev = m >= NQ
        q = m % NQ
        h = _unpack_h(res.results[m]["out"], rev)
        sl = slice(q * NC_SEQ, (q + 1) * NC_SEQ)
        if rev:
            out[sl, :, H:2 * H] = h
        else:
            out[sl, :, 0:H] = h
    return out.reshape(B, S, L, 2 * H)
